# revision 52
# baseline (speedup 1.0000x reference)
"""Trainium2 Bass kernel for the sparse-MoE block (top-2 of 8 experts).

Strategy: the router (a tiny [T,H]x[H,E] matmul + top-2) and the token
dispatch run on the host; the expert FFNs -- 99.97% of the FLOPs -- run on
8 NeuronCores. Sharding is F-parallel: each core holds a 512-wide slice of
the FFN intermediate dimension for ALL 8 experts, processes every expert's
gathered token group against its slice, and returns a partial down-proj
output. The host sums the 8 partials and scatter-adds into token order
with the routing weights. This is load-balanced regardless of routing.

All tensors are bf16 (same PE rate as fp32r, half the HBM traffic; rel
err ~4e-3 vs the 2e-2 gate). Host pre-arranges every DRAM tensor into
partition-major layout so each DMA is one contiguous run per partition.
"""

import ml_dtypes
import numpy as np

import concourse.bass as bass
import concourse.tile as tile
from concourse import mybir
from concourse.bass_utils import run_bass_kernel_spmd

B, S, H, F, E = 2, 2048, 1024, 4096, 8
TOP_K = 2
NCORES = 8
FS = F // NCORES  # 512
KO = H // 128  # 8
KF = FS // 128  # 4
F32 = mybir.dt.float32
BF16 = mybir.dt.bfloat16
NPBF16 = np.dtype(ml_dtypes.bfloat16)
SILU = mybir.ActivationFunctionType.Silu
COPY = mybir.ActivationFunctionType.Copy
MULT = mybir.AluOpType.mult


def _split_multi_waits(nc, max_waits=1):
    """This toolchain's walrus codegen supports one sync-wait per
    instruction; Tile attaches as many as needed. Hoist extras onto
    standalone NoOps just before the instruction on the same engine
    (engine streams execute in order, so semantics are preserved)."""
    total = 0
    for f in nc.m.functions:
        for bb in f.blocks:
            new_insts = []
            changed = False
            for inst in bb.instructions:
                si = inst.sync_info
                waits = list(si.on_wait) if si and si.on_wait else []
                if len(waits) > max_waits:
                    for w in waits[:-max_waits]:
                        nop = mybir.InstNoOp(
                            name=nc.get_next_instruction_name(), ins=[], outs=[]
                        )
                        nop.engine = inst.engine
                        nop.sync_info = mybir.SyncInfo(on_wait=[w], on_update=[])
                        new_insts.append(nop)
                        total += 1
                    inst.sync_info = mybir.SyncInfo(
                        on_wait=waits[-max_waits:],
                        on_update=list(si.on_update) if si.on_update else [],
                    )
                    changed = True
                new_insts.append(inst)
            if changed:
                bb.instructions = new_insts
    return total


def _expert_chunk_widths(cnt, first_narrow=False, last_narrow=False):
    # Split a token count into chunk widths <=512, keeping every chunk
    # >=256 where possible (full-rate matmuls need a reasonably wide
    # moving free dim). No padding: widths sum to cnt exactly.
    # first_narrow/last_narrow carve a 128-token chunk off the start/end
    # (shortens the DMA-starved prologue / the un-overlapped epilogue).
    if cnt == 0:
        return []
    pre, post = [], []
    if first_narrow and cnt > 576:
        pre, cnt = [320], cnt - 320
    if last_narrow and cnt > 640:
        post, cnt = [256], cnt - 256
    if cnt <= 512:
        return pre + [cnt] + post
    n512, tail = divmod(cnt, 512)
    if tail == 0:
        return pre + [512] * n512 + post
    if tail >= 256:
        return pre + [512] * n512 + [tail] + post
    # borrow from the last full chunk: 512 + tail -> (256 + tail) + 256
    return pre + [512] * (n512 - 1) + [256 + tail, 256] + post


def _make_chunks(pads):
    chunks = []
    base = 0
    for e, pad in enumerate(pads):
        off = 0
        widths = _expert_chunk_widths(
            pad, first_narrow=(e == 0), last_narrow=(e == len(pads) - 1)
        )
        for w in widths:
            chunks.append((e, base + off, w))
            off += w
        base += pad
    return chunks, base


def _build_program(pads, bufs=None, xt_eng='sync', y_eng='sync'):
    bufs = {**{'w': 3, 'x': 3, 'a': 3, 'g': 3, 'y': 6, 'pg': 2, 'pu': 3, 'py': 3}, **(bufs or {})}
    chunks, CT = _make_chunks(pads)
    nc = bass.Bass("TRN2", target_bir_lowering=False, debug=False, num_devices=NCORES)
    # All DRAM tensors are partition-major: one contiguous run per
    # partition per DMA.
    xt = nc.declare_dram_parameter("xt", [128, KO * CT], BF16, isOutput=False)
    wg = nc.declare_dram_parameter("wg", [E, 128, KO * FS], BF16, isOutput=False)
    wu = nc.declare_dram_parameter("wu", [E, 128, KO * FS], BF16, isOutput=False)
    wd = nc.declare_dram_parameter("wd", [E, 128, KF * H], BF16, isOutput=False)
    yp = nc.declare_dram_parameter("yp", [CT, H], BF16, isOutput=True)

    with tile.TileContext(nc) as tc:
        with (
            tc.tile_pool(name="wpool", bufs=bufs["w"]) as wpool,
            tc.tile_pool(name="xpool", bufs=bufs["x"]) as xpool,
            tc.tile_pool(name="apool", bufs=bufs["a"]) as apool,
            tc.tile_pool(name="gpool", bufs=bufs["g"]) as gpool,
            tc.tile_pool(name="ypool", bufs=bufs["y"]) as ypool,
            tc.tile_pool(name="pga", bufs=bufs["pg"], space="PSUM") as pg_pool,
            tc.tile_pool(name="pua", bufs=bufs["pu"], space="PSUM") as pu_pool,
            tc.tile_pool(name="pyb", bufs=bufs["py"], space="PSUM") as py_pool,
        ):

            # PE warm-up: the first ~6us after the preamble are DMA-bound
            # with the PE idle; the HAM clock gate then starts the real
            # matmul stream cold (1.2 GHz for ~3.4us). Fill the wait with
            # dummy matmuls on a scratch tile so the stream starts warm.
            warm = gpool.tile([128, 128], BF16, tag="warm")
            nc.gpsimd.memset(warm[:], 0)
            pwarm = py_pool.tile([128, 512], F32, tag="py")
            for _ in range(36):
                nc.tensor.matmul(
                    pwarm[:, :128], warm[:], warm[:], start=True, stop=True
                )

            def load_weights(e):
                # ft-major layout; split into per-ft pieces (interleaved
                # gate/up) so the first gate group only waits on 256 KB.
                # The wd load is deferred by the caller: its first use
                # (stage_b) is a chunk later, and issuing it here would
                # delay the next xt chunk on the FIFO ring.
                wgt = wpool.tile([128, KO * FS], BF16, tag="wg")
                wut = wpool.tile([128, KO * FS], BF16, tag="wu")
                wdt = wpool.tile([128, KF * H], BF16, tag="wd")
                kw = KO * 128  # cols per ft piece
                for ft in range(KF):
                    sl = slice(ft * kw, (ft + 1) * kw)
                    nc.sync.dma_start(wgt[:, sl], wg[e][:, sl])
                    nc.sync.dma_start(wut[:, sl], wu[e][:, sl])
                return wgt, wut, wdt

            def stage_b(act, w, c0, wdt, split_y=False):
                for cs in range(-(-w // 128)):
                    m = min(128, w - cs * 128)
                    yt = ypool.tile([128, H], BF16, tag="y")
                    for ht in range(2):
                        py = py_pool.tile([128, 512], F32, tag="py")
                        for kf in range(KF):
                            nc.tensor.matmul(
                                py[:m],
                                act[:, kf, cs * 128 : cs * 128 + m],
                                wdt[:, kf * H + ht * 512 : kf * H + (ht + 1) * 512],
                                start=(kf == 0),
                                stop=(kf == KF - 1),
                            )
                        # alternate copy engines so the py pool recycles at
                        # twice the single-engine copy rate
                        if ht == 0:
                            nc.vector.tensor_copy(
                                yt[:m, ht * 512 : (ht + 1) * 512], py[:m]
                            )
                        else:
                            nc.scalar.activation(
                                yt[:m, ht * 512 : (ht + 1) * 512], py[:m], COPY
                            )
                        if split_y:
                            # store each half as soon as its copy lands so
                            # the final DMA overlaps the other half's MMs.
                            getattr(nc, y_eng).dma_start(
                                yp[
                                    c0 + cs * 128 : c0 + cs * 128 + m,
                                    ht * 512 : (ht + 1) * 512,
                                ],
                                yt[:m, ht * 512 : (ht + 1) * 512],
                            )
                    if not split_y:
                        getattr(nc, y_eng).dma_start(
                            yp[c0 + cs * 128 : c0 + cs * 128 + m, :], yt[:m]
                        )

            weights = {}
            pending_wds = []
            prev = None
            for i, (e, c0, w) in enumerate(chunks):
                xtile = xpool.tile([128, KO * 512], BF16, tag="xt")
                getattr(nc, xt_eng).dma_start(
                    xtile[:, : KO * w], xt[:, KO * c0 : KO * (c0 + w)]
                )
                if pending_wds:
                    wdt_t, src = pending_wds.pop(0)
                    nc.sync.dma_start(wdt_t[:], src)
                if e not in weights:
                    weights[e] = load_weights(e)
                    pending_wds.append((weights[e][2], wd[e]))
                # Prefetch the next expert's gate/up pieces one chunk early:
                # they must lead the previous chunk's y-store issues in the
                # sync FIFO, or the boundary gate group stalls behind them.
                if i + 1 < len(chunks):
                    en = chunks[i + 1][0]
                    if en != e and en not in weights:
                        weights[en] = load_weights(en)
                        pending_wds.append((weights[en][2], wd[en]))
                wgt, wut, wdt = weights[e]
                act = apool.tile([128, KF, 512], BF16, tag="act")
                for ft in range(KF):
                    pg = pg_pool.tile([128, 512], F32, tag="pg")
                    pu = pu_pool.tile([128, 512], F32, tag="pu")

                    def gmm(dst, wt, k, start, stop):
                        nc.tensor.matmul(
                            dst[:, :w],
                            wt[:, ft * KO * 128 + k * 128 : ft * KO * 128 + (k + 1) * 128],
                            xtile[:, k * w : (k + 1) * w],
                            start=start,
                            stop=stop,
                        )

                    for k in range(KO):
                        gmm(pg, wgt, k, k == 0, k == KO - 1)
                    for k in range(KO):
                        gmm(pu, wut, k, k == 0, k == KO - 1)
                    gs = gpool.tile([128, 512], F32, tag="g")
                    nc.scalar.activation(gs[:, :w], pg[:, :w], SILU)
                    nc.vector.tensor_tensor(act[:, ft, :w], gs[:, :w], pu[:, :w], MULT)
                if prev is not None:
                    stage_b(*prev)
                prev = (act, w, c0, wdt)
            for wdt_t, src in pending_wds:
                nc.sync.dma_start(wdt_t[:], src)
            stage_b(*prev, split_y=True)

    _split_multi_waits(nc)
    return nc, CT


_program_cache = {}
LAST_RESULTS = None


def _get_program(pads):
    key = tuple(pads)
    if key not in _program_cache:
        _program_cache[key] = _build_program(pads)
    return _program_cache[key]


def _route(x, w_gate):
    """Host router: softmax(fp32) then top-2, matching jax.lax.top_k
    tie-breaking (lowest index first)."""
    logits = x @ w_gate  # [T, E] fp32
    m = logits.max(axis=-1, keepdims=True)
    p = np.exp(logits - m, dtype=np.float32)
    p /= p.sum(axis=-1, keepdims=True)
    order = np.argsort(-p, axis=-1, kind="stable")
    sel = order[:, :TOP_K]
    rw = np.take_along_axis(p, sel, axis=-1).astype(np.float32)
    return sel, rw


def kernel(hidden_states, w_gate, w_gate_proj, w_up_proj, w_down_proj):
    x = np.asarray(hidden_states, dtype=np.float32).reshape(-1, H)
    w_gate = np.asarray(w_gate, dtype=np.float32)
    WG = np.asarray(w_gate_proj, dtype=np.float32)
    WU = np.asarray(w_up_proj, dtype=np.float32)
    WD = np.asarray(w_down_proj, dtype=np.float32)
    T = x.shape[0]

    sel, rw = _route(x, w_gate)

    idx, wtok, cnts = [], [], []
    for e in range(E):
        mask0 = sel[:, 0] == e
        mask1 = sel[:, 1] == e
        ie = np.nonzero(mask0 | mask1)[0]
        idx.append(ie)
        wtok.append(np.where(mask0[ie], rw[ie, 0], rw[ie, 1]).astype(np.float32))
        cnts.append(len(ie))

    # matmuls want an even moving free dim; round each expert's token
    # count up to even (the pad column is zeros).
    ecnts = [c + (c & 1) for c in cnts]
    nc, CT = _get_program(ecnts)
    chunks, _ = _make_chunks(ecnts)

    base = np.concatenate([[0], np.cumsum(ecnts)])
    xb = np.zeros((T + 1, H), dtype=NPBF16)  # +1 zero row for pads
    xb[:T] = x.astype(NPBF16)
    # per-chunk contiguous layout [128, KO*CT]: chunk (c0,w) occupies
    # cols KO*c0 .. KO*(c0+w), ordered (ko, token) within the chunk.
    xt5 = np.zeros((128, KO * CT), dtype=NPBF16)
    for e, c0, w in chunks:
        off = c0 - base[e]
        tok = idx[e][off : off + w]
        if len(tok) < w:  # pad token(s)
            tok = np.concatenate([tok, np.full(w - len(tok), T, dtype=np.int64)])
        blk = xb[tok].T.reshape(KO, 128, w).transpose(1, 0, 2).reshape(128, KO * w)
        xt5[:, KO * c0 : KO * (c0 + w)] = blk

    # weights: partition-major per core, ft-major within the free dim.
    # [E,H,FS] -> [E,128p, KF,KO,128f] flat (col = ft*KO*128 + k*128 + j);
    # [E,FS,H] -> [E,128p, KF*H] (col = kf*H + h).
    WGb = WG.astype(NPBF16).reshape(E, KO, 128, F)
    WUb = WU.astype(NPBF16).reshape(E, KO, 128, F)
    WDb = WD.astype(NPBF16).reshape(E, NCORES, KF, 128, H)
    in_maps = []
    for c in range(NCORES):
        fsl = slice(c * FS, (c + 1) * FS)

        def wlay(Wb):
            # [E,KO,128p,FS] -> [E,KO,128p,KF,128j] -> [E,128p,KF,KO,128j]
            return np.ascontiguousarray(
                Wb[:, :, :, fsl].reshape(E, KO, 128, KF, 128).transpose(0, 2, 3, 1, 4)
            ).reshape(E, 128, KO * FS)

        in_maps.append(
            {
                "xt": xt5,
                "wg": wlay(WGb),
                "wu": wlay(WUb),
                "wd": np.ascontiguousarray(WDb[:, c].transpose(0, 2, 1, 3)).reshape(
                    E, 128, KF * H
                ),
            }
        )
    res = run_bass_kernel_spmd(nc, in_maps, list(range(NCORES)))
    global LAST_RESULTS
    LAST_RESULTS = res

    ysum = res.results[0]["yp"].astype(np.float32)
    for i in range(1, NCORES):
        ysum = ysum + res.results[i]["yp"].astype(np.float32)

    out = np.zeros((T, H), dtype=np.float32)
    for e in range(E):
        if cnts[e]:
            out[idx[e]] += ysum[base[e] : base[e] + cnts[e]] * wtok[e][:, None]
    return out.reshape(B, S, H).astype(np.float32)
